# revision 1
# baseline (speedup 1.0000x reference)
"""Multi-head attention (B=8, S=1024, D=768, H=12) on 8 TRN2 NeuronCores.

Sharding: pure batch parallelism — one batch element per core, weights
replicated. No collectives needed.

Per-core pipeline (tokens T=1024, D=768, H=12 heads of HD=64):
  1. Load x [T, D], PE-transpose to xT [D, T]; load W_qkv staged through
     SBUF and round fp32 -> tf32 (fp32r) so the PE runs at 1 cycle/row.
  2. QKV projections as fp32r matmuls:
       V [T, 768] = xT-chunks^T @ W_qkv[:, 1536:]       (stored bf16)
       Q^T, K^T [768, T] = W_qkv[:, :1536]-chunks^T @ xT (kept fp32r)
     QK chunk pairs are software-pipelined with the attention heads that
     consume them, so PE-heavy projection overlaps DVE/ACT-heavy softmax.
  3. Per head h, per query chunk qi (128 queries, causal k <= (qi+1)*128):
       s = Q_h K_h^T (fp32r), diagonal block masked with -1e10
       m = rowmax(s) on DVE; w = exp(8*s - 8*m) -> bf16 on ACT, with the
       row sums accumulated by the same ACT instruction
       w blocks PE-transposed to wT [k, q] (bf16)
       o = w @ V_h accumulated over k chunks (bf16 matmul, N=64)
       attn[:, h*64:] = o * (1/rowsum)  (stored bf16)
  4. In the last head's sweep each finished token chunk is immediately
     PE-transposed to attnT and projected: y = attn @ W_proj + b (bf16
     matmul, fp32 accumulate), then DMA'd out.

Measured vs the fp32 jax reference: rel err ~3.2e-3 on hardware (scores
and softmax stats in fp32/tf32; only w/V/attn/W_proj are bf16).
"""

import numpy as np

import concourse.bass as bass
import concourse.mybir as mybir
import concourse.tile as tile
from concourse import bacc
from concourse.bass_utils import run_bass_kernel_spmd
from concourse.masks import make_causal_mask, make_identity

B, S, D = 8, 1024, 768
H, HD = 12, 64
HV = 65  # V block width per head: 64 value cols + a ones col whose AV
         # matmul output column is the softmax denominator
NT = S // 128   # 8 token chunks
ND = D // 128   # 6 d chunks
F32 = mybir.dt.float32
F32R = mybir.dt.float32r
BF16 = mybir.dt.bfloat16

N_CORES = 8


def bank_chunks(size):
    """Split [0, size) into matmul-N chunks that each sit in one PSUM bank
    (fp32 bank = 512 elems) and are >=256 where possible (fp32r full rate)."""
    out = []
    start = 0
    while start < size:
        end = min(start + 512, size, (start // 512 + 1) * 512)
        out.append((start, end))
        start = end
    return out


def build_mha(nc):
    x_d = nc.dram_tensor("x", [S, D], F32, kind="ExternalInput")
    wqkv_d = nc.dram_tensor("W_qkv", [D, 3 * D], F32, kind="ExternalInput")
    wproj_d = nc.dram_tensor("W_proj", [D, D], F32, kind="ExternalInput")
    bproj_d = nc.dram_tensor("b_proj", [1, D], F32, kind="ExternalInput")
    out_d = nc.dram_tensor("out", [S, D], F32, kind="ExternalOutput")

    with tile.TileContext(nc) as tc:
        with (
            tc.tile_pool(name="persist", bufs=1) as pp,
            tc.tile_pool(name="psum", bufs=1, space="PSUM") as psum,
        ):
            def ptile():
                return psum.tile([128, 1024], F32, name="p1", tag="pbig", bufs=3)

            def ptiny():
                return psum.tile([128, 512], F32, name="pt1", tag="ptiny", bufs=2)

            # ---- constants ----
            ident_f32 = pp.tile([128, 128], F32, name="ident_f32", tag="ident_f32")
            make_identity(nc, ident_f32[:])
            ident_bf16 = pp.tile([128, 128], BF16, name="ident_bf16", tag="ident_bf16")
            nc.vector.tensor_copy(ident_bf16[:], ident_f32[:])
            # bigmask: [0, S) columns are zero, [S, S+128) hold the causal
            # block mask; slicing aligns the mask with the diagonal block
            bigmask = pp.tile([128, S + 128], F32, name="bigmask", tag="bigmask")
            nc.gpsimd.memset(bigmask[:], 0.0)
            make_causal_mask(nc, bigmask[:, S:S + 128], mask_val=-1e10)

            # b_proj broadcast to 128 partitions via K=1 outer product
            b_row = pp.tile([1, D], F32, name="b_row", tag="b_row")
            nc.sync.dma_start(b_row[:], bproj_d[:])
            ones_col = pp.tile([1, 128], F32, name="ones_col", tag="ones_col")
            nc.vector.memset(ones_col[:], 1.0)
            b_bcast = pp.tile([128, D], F32, name="b_bcast", tag="b_bcast")
            pb = ptile()
            for c0, c1 in bank_chunks(D):
                nc.tensor.matmul(
                    pb[:, c0:c1], ones_col[:], b_row[:, c0:c1],
                    start=True, stop=True,
                )
            nc.vector.tensor_copy(b_bcast[:], pb[:, :D])

            # ---- persistent activations ----
            qkT = [pp.tile([128, S], F32R, name=f"qkT{m}", tag=f"qkT{m}") for m in range(12)]
            v_sb = [pp.tile([128, H * HV], BF16, name=f"v{qi}", tag=f"v{qi}") for qi in range(NT)]
            attn = [pp.tile([128, D], BF16, name=f"attn{qi}", tag=f"attn{qi}") for qi in range(NT)]
            attnT = [pp.tile([128, S], BF16, name=f"attnT{di}", tag=f"attnT{di}") for di in range(ND)]
            wp = [pp.tile([128, D], BF16, name=f"wp{di}", tag=f"wp{di}") for di in range(ND)]
            for di in range(ND):
                # SWDGE cast fp32 -> bf16 during load
                nc.gpsimd.dma_start(
                    wp[di][:], wproj_d[di * 128:(di + 1) * 128, :]
                )

            with (
                tc.tile_pool(name="ph2", bufs=1) as p2,
                tc.tile_pool(name="xpool", bufs=2) as xp,
                tc.tile_pool(name="ph3", bufs=4) as p3,
                tc.tile_pool(name="ph3s", bufs=3) as p3s,
                tc.tile_pool(name="ypool", bufs=1) as yp,
            ):
                # ---- x load + transpose ----
                xT = [p2.tile([128, S], F32R, name=f"xT{di}", tag=f"xT{di}") for di in range(ND)]
                for qi in range(NT):
                    x_t = xp.tile([128, D], F32, name="x_t", tag="x_t")
                    nc.sync.dma_start(x_t[:], x_d[qi * 128:(qi + 1) * 128, :])
                    for di in range(ND):
                        pt = ptiny()
                        nc.tensor.transpose(
                            pt[:, :128], x_t[:, di * 128:(di + 1) * 128],
                            ident_f32[:]
                        )
                        nc.vector.tensor_copy(
                            xT[di][:, qi * 128:(qi + 1) * 128], pt[:, :128]
                        )

                # ---- W_qkv load + tf32 rounding, V columns first so the V
                # projection (and then attention) can start early ----
                wq = [p2.tile([128, 3 * D], F32R, name=f"wq{di}", tag=f"wq{di}") for di in range(ND)]
                n_stage = 0
                for part in (2, 0, 1):
                    for di in range(ND):
                        tag = "x_t" if n_stage % 2 == 0 else "y_t"
                        pool = xp if n_stage % 2 == 0 else yp
                        wq_stage = pool.tile([128, D], F32, name="wq_stage", tag=tag)
                        eng = nc.sync if n_stage % 2 == 0 else nc.scalar
                        n_stage += 1
                        eng.dma_start(
                            wq_stage[:],
                            wqkv_d[di * 128:(di + 1) * 128,
                                   part * D:(part + 1) * D],
                        )
                        # rounds fp32 -> tf32 (fp32r) for full-rate PE use
                        nc.vector.tensor_copy(
                            wq[di][:, part * D:(part + 1) * D], wq_stage[:]
                        )

                # ---- V in [token, dv] layout, with ones column per head ----
                for qi in range(NT):
                    pv = ptile()
                    for c0, c1 in bank_chunks(D):
                        for di in range(ND):
                            nc.tensor.matmul(
                                pv[:, c0:c1],
                                xT[di][:, qi * 128:(qi + 1) * 128],
                                wq[di][:, 2 * D + c0:2 * D + c1],
                                start=(di == 0), stop=(di == ND - 1),
                            )
                    nc.gpsimd.memset(
                        v_sb[qi][:].rearrange("p (h v) -> p h v", v=HV)[:, :, HD:], 1.0
                    )
                    nc.vector.tensor_copy(
                        v_sb[qi][:].rearrange("p (h v) -> p h v", v=HV)[:, :, :HD],
                        pv[:, :D].rearrange("p (h v) -> p h v", v=HD),
                    )

                def qk_chunk(m):
                    pqk = ptile()
                    for c0, c1 in bank_chunks(S):
                        for di in range(ND):
                            nc.tensor.matmul(
                                pqk[:, c0:c1],
                                wq[di][:, m * 128:(m + 1) * 128],
                                xT[di][:, c0:c1],
                                start=(di == 0), stop=(di == ND - 1),
                            )
                    if m < 6:
                        # pre-scale Q by 8: scores then come out as 8*s and
                        # the softmax needs no separate x8 pass
                        nc.scalar.mul(qkT[m][:], pqk[:], 8.0)
                    else:
                        nc.vector.tensor_copy(qkT[m][:], pqk[:])

                def attention_head(h):
                    qoff = (h % 2) * 64
                    Qt = qkT[h // 2]
                    Kt = qkT[6 + h // 2]
                    for qi in range(NT):
                        ks = (qi + 1) * 128
                        lhs = Qt[qoff:qoff + 64, qi * 128:(qi + 1) * 128]
                        ps = ptile()
                        for c0, c1 in bank_chunks(ks):
                            nc.tensor.matmul(
                                ps[:, c0:c1],
                                lhs,
                                Kt[qoff:qoff + 64, c0:c1],
                                start=True, stop=True,
                            )
                        # causal mask on the diagonal block
                        nc.vector.tensor_tensor(
                            out=ps[:, qi * 128:ks],
                            in0=ps[:, qi * 128:ks],
                            in1=bigmask[:, S:S + 128],
                            op=mybir.AluOpType.add,
                        )
                        neg8m = p3s.tile([128, 1], F32, name="neg8m", tag="neg8m", bufs=6)
                        nc.vector.reduce_max(
                            out=neg8m[:], in_=ps[:, :ks],
                            axis=mybir.AxisListType.X, negate=True,
                        )
                        w_t = p3s.tile([128, S], BF16, name="w_t", tag="w_t")
                        nc.scalar.activation(
                            w_t[:, :ks], ps[:, :ks],
                            mybir.ActivationFunctionType.Exp,
                            bias=neg8m[:], scale=1.0,
                        )

                        # w[q, ki*128:] -> wT[:, ki, q] via PE transpose;
                        # all blocks stage through one 1-bank psum tile and
                        # evacuate in a single ACT copy
                        wT = p3.tile([128, NT, 128], BF16, name="wT", tag="wT")
                        pt8 = ptiny().bitcast(BF16).rearrange(
                            "p (b q) -> p b q", q=128
                        )
                        for ki in range(qi + 1):
                            nc.tensor.transpose(
                                pt8[:, ki, :],
                                w_t[:, ki * 128:(ki + 1) * 128],
                                ident_bf16[:],
                            )
                        nc.scalar.copy(
                            wT[:, :qi + 1, :], pt8[:, :qi + 1, :]
                        )

                        # o = w @ [V_h | 1]; last column = softmax denominator
                        po = ptiny()
                        for ki in range(qi + 1):
                            nc.tensor.matmul(
                                po[:, :HV],
                                wT[:, ki, :],
                                v_sb[ki][:, h * HV:(h + 1) * HV],
                                start=(ki == 0), stop=(ki == qi),
                            )
                        recip = p3s.tile([128, 1], F32, name="recip", tag="recip", bufs=6)
                        nc.vector.reciprocal(recip[:], po[:, HD:HV])
                        nc.vector.tensor_scalar_mul(
                            attn[qi][:, h * HD:(h + 1) * HD],
                            po[:, :HD],
                            recip[:],
                        )

                        if h == H - 1:
                            # all heads done for token chunk qi: project now
                            for di0 in range(0, ND, 4):
                                nb = min(4, ND - di0)
                                pat = ptiny().bitcast(BF16).rearrange(
                                    "p (b q) -> p b q", q=128
                                )
                                for j in range(nb):
                                    di = di0 + j
                                    nc.tensor.transpose(
                                        pat[:, j, :],
                                        attn[qi][:, di * 128:(di + 1) * 128],
                                        ident_bf16[:],
                                    )
                                for j in range(nb):
                                    di = di0 + j
                                    nc.scalar.copy(
                                        attnT[di][:, qi * 128:(qi + 1) * 128],
                                        pat[:, j, :],
                                    )
                            y_t = yp.tile([128, D], F32, name="y_t", tag="y_t")
                            py = ptile()
                            for c0, c1 in bank_chunks(D):
                                for di in range(ND):
                                    nc.tensor.matmul(
                                        py[:, c0:c1],
                                        attnT[di][:, qi * 128:(qi + 1) * 128],
                                        wp[di][:, c0:c1],
                                        start=(di == 0), stop=(di == ND - 1),
                                    )
                            nc.vector.tensor_tensor(
                                out=y_t[:], in0=py[:, :D], in1=b_bcast[:],
                                op=mybir.AluOpType.add,
                            )
                            nc.sync.dma_start(
                                out_d[qi * 128:(qi + 1) * 128, :], y_t[:]
                            )

                # software pipeline: each QK chunk pair immediately feeds the
                # two heads that consume it, so PE-heavy QK overlaps the
                # DVE/ACT-heavy softmax of previous heads
                for r in range(6):
                    qk_chunk(r)
                    qk_chunk(6 + r)
                    attention_head(2 * r)
                    attention_head(2 * r + 1)

    nc.compile()
    return nc


_NC_CACHE = None


def _get_nc():
    global _NC_CACHE
    if _NC_CACHE is None:
        nc = bacc.Bacc(
            "TRN2",
            target_bir_lowering=False,
            debug=False,
            num_devices=N_CORES,
        )
        build_mha(nc)
        _NC_CACHE = nc
    return _NC_CACHE


def kernel(x, W_qkv, W_proj, b_proj):
    nc = _get_nc()
    x = np.ascontiguousarray(np.asarray(x, dtype=np.float32))
    W_qkv = np.ascontiguousarray(np.asarray(W_qkv, dtype=np.float32))
    W_proj = np.ascontiguousarray(np.asarray(W_proj, dtype=np.float32))
    b_proj = np.ascontiguousarray(
        np.asarray(b_proj, dtype=np.float32).reshape(1, D)
    )
    in_maps = [
        {"x": x[b], "W_qkv": W_qkv, "W_proj": W_proj, "b_proj": b_proj}
        for b in range(N_CORES)
    ]
    res = run_bass_kernel_spmd(nc, in_maps, core_ids=list(range(N_CORES)))
    return np.stack([res.results[b]["out"] for b in range(N_CORES)], axis=0)

